# revision 39
# baseline (speedup 1.0000x reference)
"""NetVLAD pooling kernel for Trainium2 (8 NeuronCores, data-parallel over B).

Math (per batch row b):
    logits = feats @ assign_w.T              # (L, K); assign_b cancels in softmax over L
    a_u    = exp(logits + maskbias)          # maskbias = -1e30 for l >= lens[b]
    U      = a_u.T @ feats                   # (K, D) unnormalized
    s      = sum_l a_u[l, :]                 # (K,)
    vlad   = U / s - centroids               # host
    out    = l2norm(vlad.min(axis=0))        # host

Device structure (per core: 4 batch rows, python-unrolled, software-pipelined
with per-tile stage skews; stages are list-scheduled by the Tile framework):

  HBM traffic per core (~33 MiB): the NATURAL bf16 layout of feats (pass B
  operand, 32 MiB) plus only the LAST D-chunk pre-transposed to fp8 on the
  host (1 MiB, HYB=1); the other 7 transposed chunks needed by the logits
  matmul are produced on-chip so HBM stays near the single-layout floor.

  per 128-token tile t:
    T:  PE transposes 7 natural chunks [128L,128D] into one PSUM bank
        (bf16, one shared accumulation group; is_transpose matmuls)
    CB: PSUM -> SBUF fp8-e4m3 copyback split ACT (first 328 cols) / DVE
        (rest) -- gpsimd cannot read PSUM on real HW
    A:  logits psum_lg[128L,64K]: 3 DoubleRow fp8 matmuls (PE chunk pairs)
        + 2 plain fp8 matmuls (PE chunk 6, HBM chunk 7; the HBM one is
        issued first since it never waits on the copyback, keeping the
        4-deep PE wait queue from blocking the sequencer)
    E:  a_u = Exp(psum_lg + mask_col) -> bf16 (ACT; per-partition bias;
        two consecutive tiles share one psum bank, doubling buffering)
    B:  U.T accumulation: psum_u[:, di*64:+64] += nat[:, di*128:+128].T @ a_u
        (out free dim 64 per matmul halves PE cost vs the U layout; split
        into 4+4 chunks at different skews so waits fit the PE wait queue)
    s:  psum_s[64,1] += a_u.T @ ones

  Output per row: [128, 513] f32 = U.T (8 chunks x 64K) | s; host finishes
  vlad = U/s - centroids, min over K, l2 normalize.

  Modeled (TimelineSim) 113.2 us/core (baseline 198.4); measured rel err
  6.3e-3 against the f32 reference (gate 2e-2).
"""

import numpy as np

import concourse.bass as bass
import concourse.mybir as mybir
import concourse.tile as tile
from concourse import bacc
from concourse.bass_utils import run_bass_kernel_spmd

B, L, D, K = 32, 4096, 1024, 64
NCORES = 8
BPC = B // NCORES          # batch rows per core
F32 = mybir.dt.float32
BF16 = mybir.dt.bfloat16
FP8 = mybir.dt.float8e4    # e4m3: pass-A operands (weight noise ~ 6e-3 final err)

import os
NSEG = int(os.environ.get("K_NSEG", "16"))  # L-segments per row (one DMA each)
FBUFS = int(os.environ.get("K_FBUFS", "9"))   # natural-segment prefetch depth
PST_BUFS = int(os.environ.get("K_PST", "4"))
PSU_BUFS = int(os.environ.get("K_PSU", "1"))
SKEWS = tuple(int(x) for x in os.environ.get("K_SKEWS", "4,6,7").split(","))
PSL_BUFS = int(os.environ.get("K_PSL", "2"))
LG_PAIR = int(os.environ.get("K_LGPAIR", "1"))
HYB = int(os.environ.get("K_HYB", "1"))    # chunks loaded fp8-transposed from HBM
ALTQ = int(os.environ.get("K_ALTQ", "0"))  # alternate seg loads sync/scalar
ASPLIT = int(os.environ.get("K_ASPLIT", "328"))  # copyback cols on ACT (rest DVE)
def build_kernel(bpc=BPC, l=L, d=D, k=K, nseg=NSEG, fbufs=FBUFS):
    """Build + compile the per-core module. All 8 cores run the same module."""
    lt = l // 128            # L-tiles per row (32)
    dc = d // 128            # 128-wide D chunks (8)
    seg_l = l // nseg        # tokens per segment
    spt = seg_l // 128       # L-tiles per segment

    nc = bacc.Bacc(None, target_bir_lowering=False, debug=False)
    fnat_hbm = nc.dram_tensor("fnat", [bpc, nseg, 128, spt * d], BF16,
                              kind="ExternalInput")
    # fp8 transposed LAST HYB chunks, row-granular: for tile t, chunk c:
    # [b, p, t*HYB*128 + c*128 + l] = feats[b, t*128+l, (dc-HYB+c)*128+p]
    ft7_hbm = (nc.dram_tensor("ft7", [bpc, 128, lt * HYB * 128], FP8,
                              kind="ExternalInput") if HYB else None)
    wt_hbm = nc.dram_tensor("wt", [128, dc * k], FP8, kind="ExternalInput")
    mask_hbm = nc.dram_tensor("mask_t", [128, bpc * lt], F32, kind="ExternalInput")
    ident_hbm = nc.dram_tensor("ident", [128, 128], BF16, kind="ExternalInput")
    out_us = nc.dram_tensor("out_us", [bpc, 128, dc * k + 1], BF16,
                            kind="ExternalOutput")

    with tile.TileContext(nc) as tc:
        with (
            tc.tile_pool(name="consts", bufs=1) as consts,
            tc.tile_pool(name="fseg", bufs=fbufs) as fpool,
            tc.tile_pool(name="ft7r", bufs=2) as ft7pool,
            tc.tile_pool(name="ft", bufs=int(os.environ.get("K_FTB", "8"))) as ftpool,
            tc.tile_pool(name="au", bufs=int(os.environ.get("K_AUB", "8"))) as aupool,
            tc.tile_pool(name="outs", bufs=4) as outpool,
            tc.tile_pool(name="psT", bufs=PST_BUFS, space="PSUM") as psT,
            tc.tile_pool(name="psL", bufs=PSL_BUFS, space="PSUM") as psL,
            tc.tile_pool(name="psU", bufs=PSU_BUFS, space="PSUM") as psU,
            tc.tile_pool(name="psS", bufs=1, space="PSUM") as psS,
        ):
            # ident first (tile-0 transposes need it); wt/mask are only
            # needed from pass A / exp onward, so their loads are emitted
            # after the first segment DMA (see below) to start the feats
            # stream earlier
            ident = consts.tile([128, 128], BF16)
            nc.gpsimd.dma_start(out=ident, in_=ident_hbm[:])
            wt_sb = consts.tile([128, dc * k], FP8)
            mask_sb = consts.tile([128, bpc * lt], F32)
            ones = consts.tile([128, 1], BF16)
            nc.vector.memset(ones, 1.0)

            ntiles = bpc * lt
            state = {}              # per-tile carry: nat/ft/a_u APs
            rows = {}               # per-row psum_u / psum_s

            def stage1(tg):
                """DMA (at segment boundary), transposes, copyback halves."""
                b, t = divmod(tg, lt)
                sg, j = divmod(t, spt)
                if HYB and t == 0:
                    ft7row = ft7pool.tile([128, lt * HYB * 128], FP8,
                                          name="ft7row")
                    nc.sync.dma_start(out=ft7row, in_=ft7_hbm[b])
                    state["ft7row"] = ft7row
                if j == 0:
                    fseg = fpool.tile([128, spt * d], BF16, name="fseg")
                    q = nc.scalar if (ALTQ and sg % 2) else nc.sync
                    q.dma_start(out=fseg, in_=fnat_hbm[b, sg])
                    state["fseg"] = fseg
                nat = state["fseg"][:, j * d:(j + 1) * d]
                ft7 = (state["ft7row"][:, t * HYB * 128:(t + 1) * HYB * 128]
                       if HYB else None)
                ntr = dc - HYB                # chunks transposed on PE
                pT = psT.tile([128, ntr * 128], BF16, name="pT")
                for di in range(ntr):
                    nc.tensor.matmul(
                        pT[:, di * 128:(di + 1) * 128],
                        nat[:, di * 128:(di + 1) * 128],
                        ident, is_transpose=True,
                        start=(di == 0), stop=(di == ntr - 1),
                        skip_group_check=True,
                    )
                ft = ftpool.tile([128, ntr * 128], FP8, name="ft")
                asp = min(ASPLIT, ntr * 128)
                nc.scalar.copy(ft[:, 0:asp], pT[:, 0:asp])
                if asp < ntr * 128:
                    nc.vector.tensor_copy(ft[:, asp:ntr * 128],
                                          pT[:, asp:ntr * 128])
                state[("nat", tg)] = nat
                state[("ft", tg)] = ft
                state[("ft7", tg)] = ft7

            def stage2(tg):
                """Pass A (DoubleRow fp8) + exp. Two tiles share one psum bank."""
                b, t = divmod(tg, lt)
                ft = state.pop(("ft", tg))
                ft7 = state.pop(("ft7", tg))
                if LG_PAIR:
                    if tg % 2 == 0:
                        state["lg2"] = psL.tile([128, 2 * k], F32, name="lg")
                    psum_lg = state["lg2"][:, (tg % 2) * k:(tg % 2 + 1) * k]
                    lg_first = tg % 2 == 0
                else:
                    psum_lg = psL.tile([128, k], F32, name="lg")
                    lg_first = True
                ntr = dc - HYB
                # HBM chunks first: they only wait on the row DMA, so the
                # copyback-dependent matmuls that follow fit the 4-slot PE
                # wait queue (no sequencer head-of-line block)
                for jj in range(HYB // 2):       # DR pairs within ft7
                    nc.tensor.matmul(
                        psum_lg,
                        ft7[:, jj * 256:(jj + 1) * 256]
                        .rearrange("p (two m) -> p two m", two=2),
                        wt_sb[:, (ntr + 2 * jj) * k:(ntr + 2 * jj + 2) * k]
                        .rearrange("p (two n) -> p two n", two=2),
                        start=(lg_first and jj == 0), stop=False,
                        perf_mode=mybir.MatmulPerfMode.DoubleRow,
                        skip_group_check=True,
                    )
                if HYB % 2:                      # leftover single HBM chunk
                    nc.tensor.matmul(
                        psum_lg, ft7[:, (HYB - 1) * 128:HYB * 128],
                        wt_sb[:, (dc - 1) * k:dc * k],
                        start=(lg_first and HYB // 2 == 0), stop=False,
                        skip_group_check=True,
                    )
                for j2 in range(ntr // 2):       # DR pairs over PE chunks
                    nc.tensor.matmul(
                        psum_lg,
                        ft[:, j2 * 256:(j2 + 1) * 256]
                        .rearrange("p (two m) -> p two m", two=2),
                        wt_sb[:, j2 * 128:(j2 + 1) * 128]
                        .rearrange("p (two n) -> p two n", two=2),
                        start=(HYB == 0 and lg_first and j2 == 0),
                        stop=(ntr % 2 == 0 and j2 == ntr // 2 - 1),
                        perf_mode=mybir.MatmulPerfMode.DoubleRow,
                        skip_group_check=True,
                    )
                if ntr % 2:                      # leftover single PE chunk
                    nc.tensor.matmul(
                        psum_lg, ft[:, (ntr - 1) * 128:ntr * 128],
                        wt_sb[:, (ntr - 1) * k:ntr * k],
                        start=False, stop=True, skip_group_check=True,
                    )
                a_u = aupool.tile([128, k], BF16, name="au")
                nc.scalar.activation(
                    a_u, psum_lg, mybir.ActivationFunctionType.Exp,
                    bias=mask_sb[:, b * lt + t:b * lt + t + 1],
                )
                state[("au", tg)] = a_u

            def stage3(tg, chunks):
                """Pass B (U.T accumulation) for a subset of D chunks (+ s)."""
                b, t = divmod(tg, lt)
                if t == 0 and chunks[0] == 0:
                    rows[b] = (psU.tile([128, dc * k], F32, name="u"),
                               psS.tile([64, 1], F32, name="s"))
                psum_u, psum_s = rows[b]
                nat = state[("nat", tg)]
                a_u = state[("au", tg)]
                for di in chunks:
                    nc.tensor.matmul(
                        psum_u[:, di * k:(di + 1) * k],
                        nat[:, di * 128:(di + 1) * 128],
                        a_u,
                        start=(t == 0 and di == 0),
                        stop=(t == lt - 1 and di == dc - 1),
                        skip_group_check=True,
                    )
                if chunks[-1] == dc - 1:
                    nc.tensor.matmul(
                        psum_s, a_u, ones,
                        start=(t == 0), stop=(t == lt - 1),
                    )
                    state.pop(("nat", tg))
                    state.pop(("au", tg))
                    if t == lt - 1:
                        us_sb = outpool.tile([128, dc * k + 1], BF16, name="us")
                        hk = dc * k // 2
                        nc.scalar.copy(us_sb[:, 0:hk], psum_u[:, 0:hk])
                        nc.vector.tensor_copy(us_sb[:, hk:dc * k],
                                              psum_u[:, hk:dc * k])
                        nc.vector.memset(us_sb[64:128, dc * k:dc * k + 1], 0.0)
                        nc.vector.tensor_copy(us_sb[0:64, dc * k:dc * k + 1],
                                              psum_s)
                        nc.gpsimd.dma_start(out=out_us[b], in_=us_sb)

            S2, S3A, S3B = SKEWS
            lo = list(range(dc // 2))
            hi = list(range(dc // 2, dc))
            for tg in range(ntiles + S3B):
                if tg < ntiles:
                    stage1(tg)
                if tg == 0:
                    nc.scalar.dma_start(out=wt_sb, in_=wt_hbm[:])
                    nc.scalar.dma_start(out=mask_sb, in_=mask_hbm[:])
                if 0 <= tg - S2 < ntiles:
                    stage2(tg - S2)
                if 0 <= tg - S3A < ntiles:
                    stage3(tg - S3A, lo)
                if 0 <= tg - S3B < ntiles:
                    stage3(tg - S3B, hi)
    nc.compile()
    return nc


_NC_CACHE = {}


def _get_nc():
    key = (NSEG, FBUFS, PST_BUFS, PSU_BUFS, PSL_BUFS, LG_PAIR, SKEWS, HYB, ALTQ, ASPLIT,
           os.environ.get("K_FTB", "8"), os.environ.get("K_AUB", "8"))
    if key not in _NC_CACHE:
        _NC_CACHE[key] = build_kernel()
    return _NC_CACHE[key]


def pack_host_inputs(feats, lens, assign_w, bpc=BPC, l=L, d=D, k=K, nseg=NSEG):
    """Host-side sharding + packing. Returns per-core input dicts."""
    np_bf16 = mybir.dt.np(BF16)
    np_fp8 = mybir.dt.np(FP8)
    lt = l // 128
    dc = d // 128
    seg_l = l // nseg
    spt = seg_l // 128

    wt_host = np.ascontiguousarray(assign_w.T).reshape(dc, 128, k)
    wt_p = np.ascontiguousarray(
        wt_host.transpose(1, 0, 2).reshape(128, dc * k)).astype(np_fp8)
    ident = np.eye(128, dtype=np_bf16)

    in_maps = []
    for i in range(NCORES):
        rows = feats[i * bpc:(i + 1) * bpc]                      # (bpc, L, D) f32
        b16 = rows.astype(np_bf16)
        # natural: [b, sg, p, jt*d + dd] = feats[b, sg*seg_l + jt*128 + p, dd]
        fnat = b16.reshape(bpc, nseg, spt, 128, d).transpose(0, 1, 3, 2, 4)
        fnat = np.ascontiguousarray(fnat).reshape(bpc, nseg, 128, spt * d)
        # transposed fp8 last HYB chunks, row-granular:
        # [b, p, t*HYB*128 + c*128 + ll] = feats[b, t*128+ll, (dc-HYB+c)*128+p]
        f7 = rows[:, :, (dc - HYB) * 128:].astype(np_fp8)        # (bpc, L, HYB*128)
        f7 = f7.reshape(bpc, lt, 128, HYB, 128).transpose(0, 4, 1, 3, 2)
        f7 = np.ascontiguousarray(f7).reshape(bpc, 128, lt * HYB * 128)

        lens_core = lens[i * bpc:(i + 1) * bpc]
        pos = (np.arange(lt)[None, :, None] * 128
               + np.arange(128)[None, None, :])                  # (1, lt, 128)
        m = np.where(pos < lens_core[:, None, None], 0.0, -1e30).astype(np.float32)
        mask_t = np.ascontiguousarray(m.transpose(2, 0, 1).reshape(128, bpc * lt))

        im = {
            "fnat": fnat,
            "wt": wt_p,
            "mask_t": mask_t,
            "ident": ident,
        }
        if HYB:
            im["ft7"] = f7
        in_maps.append(im)
    return in_maps


def unpack_output(results, centroids, bpc=BPC, d=D, k=K):
    dc = d // 128
    out = np.empty((B, D), dtype=np.float32)
    for i in range(NCORES):
        us = results[i]["out_us"].astype(np.float32)   # (bpc, 128, dc*k+1)
        ut = us[:, :, 0:dc * k].reshape(bpc, 128, dc, k)
        u = np.ascontiguousarray(ut.transpose(0, 3, 2, 1)).reshape(bpc, k, d)
        s = us[:, 0:k, dc * k]               # (bpc, k)
        vlad = u / s[:, :, None] - centroids[None, :, :]
        o = vlad.min(axis=1)                 # (bpc, D)
        n = np.maximum(np.linalg.norm(o, axis=-1, keepdims=True), 1e-12)
        out[i * bpc:(i + 1) * bpc] = o / n
    return out


def kernel(feats, lens, assign_w, assign_b, centroids):
    feats = np.asarray(feats, dtype=np.float32)
    lens = np.asarray(lens, dtype=np.int32)
    assign_w = np.asarray(assign_w, dtype=np.float32)
    centroids = np.asarray(centroids, dtype=np.float32)

    nc = _get_nc()
    in_maps = pack_host_inputs(feats, lens, assign_w)
    # transient device errors (NRT_EXEC_UNIT_UNRECOVERABLE) recover on retry
    last_exc = None
    for _ in range(3):
        try:
            res = run_bass_kernel_spmd(nc, in_maps, core_ids=list(range(NCORES)))
            break
        except Exception as e:  # noqa: BLE001
            last_exc = e
    else:
        raise last_exc

    return unpack_output(res.results, centroids)



# revision 42
# speedup vs baseline: 1.0038x; 1.0038x over previous
"""NetVLAD pooling kernel for Trainium2 (8 NeuronCores, data-parallel over B).

Math (per batch row b):
    logits = feats @ assign_w.T              # (L, K); assign_b cancels in softmax over L
    a_u    = exp(logits + maskbias)          # maskbias = -1e30 for l >= lens[b]
    U      = a_u.T @ feats                   # (K, D) unnormalized
    s      = sum_l a_u[l, :]                 # (K,)
    vlad   = U / s - centroids               # host
    out    = l2norm(vlad.min(axis=0))        # host

Device structure (per core: 4 batch rows, python-unrolled, software-pipelined
with per-tile stage skews; stages are list-scheduled by the Tile framework):

  HBM traffic per core (~33 MiB): the NATURAL bf16 layout of feats (pass B
  operand, 32 MiB) plus only the LAST D-chunk pre-transposed to fp8 on the
  host (1 MiB, HYB=1); the other 7 transposed chunks needed by the logits
  matmul are produced on-chip so HBM stays near the single-layout floor.

  per 128-token tile t:
    T:  PE transposes 7 natural chunks [128L,128D] into one PSUM bank
        (bf16, one shared accumulation group; is_transpose matmuls)
    CB: PSUM -> SBUF fp8-e4m3 copyback split ACT (first 328 cols) / DVE
        (rest) -- gpsimd cannot read PSUM on real HW
    A:  logits psum_lg[128L,64K]: 3 DoubleRow fp8 matmuls (PE chunk pairs)
        + 2 plain fp8 matmuls (PE chunk 6, HBM chunk 7; the HBM one is
        issued first since it never waits on the copyback, keeping the
        4-deep PE wait queue from blocking the sequencer)
    E:  a_u = Exp(psum_lg + mask_col) -> bf16 (ACT; per-partition bias;
        two consecutive tiles share one psum bank, doubling buffering)
    B:  U.T accumulation: psum_u[:, di*64:+64] += nat[:, di*128:+128].T @ a_u
        (out free dim 64 per matmul halves PE cost vs the U layout; split
        into 4+4 chunks at different skews so waits fit the PE wait queue)
    s:  psum_s[64,1] += a_u.T @ ones

  Output per row: [128, 513] f32 = U.T (8 chunks x 64K) | s; host finishes
  vlad = U/s - centroids, min over K, l2 normalize.

  Modeled (TimelineSim) 113.2 us/core (baseline 198.4); measured rel err
  6.3e-3 against the f32 reference (gate 2e-2).
"""

import numpy as np

import concourse.bass as bass
import concourse.mybir as mybir
import concourse.tile as tile
from concourse import bacc
from concourse.bass_utils import run_bass_kernel_spmd

B, L, D, K = 32, 4096, 1024, 64
NCORES = 8
BPC = B // NCORES          # batch rows per core
F32 = mybir.dt.float32
BF16 = mybir.dt.bfloat16
FP8 = mybir.dt.float8e4    # e4m3: pass-A operands (weight noise ~ 6e-3 final err)

import os
NSEG = int(os.environ.get("K_NSEG", "16"))  # L-segments per row (one DMA each)
FBUFS = int(os.environ.get("K_FBUFS", "9"))   # natural-segment prefetch depth
PST_BUFS = int(os.environ.get("K_PST", "4"))
PSU_BUFS = int(os.environ.get("K_PSU", "1"))
SKEWS = tuple(int(x) for x in os.environ.get("K_SKEWS", "4,6,7").split(","))
PSL_BUFS = int(os.environ.get("K_PSL", "2"))
LG_PAIR = int(os.environ.get("K_LGPAIR", "1"))
HYB = int(os.environ.get("K_HYB", "1"))    # chunks loaded fp8-transposed from HBM
ALTQ = int(os.environ.get("K_ALTQ", "0"))  # alternate seg loads sync/scalar
ASPLIT = int(os.environ.get("K_ASPLIT", "328"))  # copyback cols on ACT (rest DVE)
LASTROW2 = int(os.environ.get("K_LASTROW", "0"))  # last row: chunks 6+7 from HBM (regressed; keep off)
def build_kernel(bpc=BPC, l=L, d=D, k=K, nseg=NSEG, fbufs=FBUFS):
    """Build + compile the per-core module. All 8 cores run the same module."""
    lt = l // 128            # L-tiles per row (32)
    dc = d // 128            # 128-wide D chunks (8)
    seg_l = l // nseg        # tokens per segment
    spt = seg_l // 128       # L-tiles per segment

    nc = bacc.Bacc(None, target_bir_lowering=False, debug=False)
    fnat_hbm = nc.dram_tensor("fnat", [bpc, nseg, 128, spt * d], BF16,
                              kind="ExternalInput")
    # fp8 transposed LAST HYB chunks, row-granular: for tile t, chunk c:
    # [b, p, t*HYB*128 + c*128 + l] = feats[b, t*128+l, (dc-HYB+c)*128+p]
    ft7_hbm = (nc.dram_tensor("ft7", [bpc, 128, lt * HYB * 128], FP8,
                              kind="ExternalInput") if HYB else None)
    # last row only: chunks 6 AND 7 fp8-transposed, DR-pairable, loaded in
    # the end-of-stream DMA slack to cut the drain-phase per-tile cost
    ft67_hbm = (nc.dram_tensor("ft67", [128, lt * 256], FP8,
                               kind="ExternalInput") if LASTROW2 else None)
    wt_hbm = nc.dram_tensor("wt", [128, dc * k], FP8, kind="ExternalInput")
    mask_hbm = nc.dram_tensor("mask_t", [128, bpc * lt], F32, kind="ExternalInput")
    ident_hbm = nc.dram_tensor("ident", [128, 128], BF16, kind="ExternalInput")
    out_us = nc.dram_tensor("out_us", [bpc, 128, dc * k + 1], BF16,
                            kind="ExternalOutput")

    with tile.TileContext(nc) as tc:
        with (
            tc.tile_pool(name="consts", bufs=1) as consts,
            tc.tile_pool(name="fseg", bufs=fbufs) as fpool,
            tc.tile_pool(name="ft7r", bufs=2) as ft7pool,
            tc.tile_pool(name="ft", bufs=int(os.environ.get("K_FTB", "8"))) as ftpool,
            tc.tile_pool(name="au", bufs=int(os.environ.get("K_AUB", "8"))) as aupool,
            tc.tile_pool(name="outs", bufs=4) as outpool,
            tc.tile_pool(name="psT", bufs=PST_BUFS, space="PSUM") as psT,
            tc.tile_pool(name="psL", bufs=PSL_BUFS, space="PSUM") as psL,
            tc.tile_pool(name="psU", bufs=PSU_BUFS, space="PSUM") as psU,
            tc.tile_pool(name="psS", bufs=1, space="PSUM") as psS,
        ):
            # ident first (tile-0 transposes need it); wt/mask are only
            # needed from pass A / exp onward, so their loads are emitted
            # after the first segment DMA (see below) to start the feats
            # stream earlier
            ident = consts.tile([128, 128], BF16)
            nc.gpsimd.dma_start(out=ident, in_=ident_hbm[:])
            wt_sb = consts.tile([128, dc * k], FP8)
            mask_sb = consts.tile([128, bpc * lt], F32)
            ones = consts.tile([128, 1], BF16)
            nc.vector.memset(ones, 1.0)

            ntiles = bpc * lt
            state = {}              # per-tile carry: nat/ft/a_u APs
            rows = {}               # per-row psum_u / psum_s

            def stage1(tg):
                """DMA (at segment boundary), transposes, copyback halves."""
                b, t = divmod(tg, lt)
                sg, j = divmod(t, spt)
                last2 = LASTROW2 and b == bpc - 1
                if HYB and t == 0 and not last2:
                    ft7row = ft7pool.tile([128, lt * HYB * 128], FP8,
                                          name="ft7row")
                    nc.sync.dma_start(out=ft7row, in_=ft7_hbm[b])
                    state["ft7row"] = ft7row
                if j == 0:
                    fseg = fpool.tile([128, spt * d], BF16, name="fseg")
                    q = nc.scalar if (ALTQ and sg % 2) else nc.sync
                    q.dma_start(out=fseg, in_=fnat_hbm[b, sg])
                    state["fseg"] = fseg
                if last2 and t == 0:
                    ft67 = ft7pool.tile([128, lt * 256], FP8, name="ft67",
                                        bufs=1)
                    nc.sync.dma_start(out=ft67, in_=ft67_hbm[:])
                    state["ft67"] = ft67
                nat = state["fseg"][:, j * d:(j + 1) * d]
                if last2:
                    ft7 = state["ft67"][:, t * 256:(t + 1) * 256]
                elif HYB:
                    ft7 = state["ft7row"][:, t * HYB * 128:(t + 1) * HYB * 128]
                else:
                    ft7 = None
                ntr = (dc - 2) if last2 else (dc - HYB)  # PE-transposed chunks
                pT = psT.tile([128, ntr * 128], BF16, name="pT")
                for di in range(ntr):
                    nc.tensor.matmul(
                        pT[:, di * 128:(di + 1) * 128],
                        nat[:, di * 128:(di + 1) * 128],
                        ident, is_transpose=True,
                        start=(di == 0), stop=(di == ntr - 1),
                        skip_group_check=True,
                    )
                ft = ftpool.tile([128, ntr * 128], FP8, name="ft")
                asp = min(ASPLIT, ntr * 128)
                nc.scalar.copy(ft[:, 0:asp], pT[:, 0:asp])
                if asp < ntr * 128:
                    nc.vector.tensor_copy(ft[:, asp:ntr * 128],
                                          pT[:, asp:ntr * 128])
                state[("nat", tg)] = nat
                state[("ft", tg)] = ft
                state[("ft7", tg)] = ft7
                state[("last2", tg)] = last2

            def stage2(tg):
                """Pass A (DoubleRow fp8) + exp. Two tiles share one psum bank."""
                b, t = divmod(tg, lt)
                ft = state.pop(("ft", tg))
                ft7 = state.pop(("ft7", tg))
                if LG_PAIR:
                    if tg % 2 == 0:
                        state["lg2"] = psL.tile([128, 2 * k], F32, name="lg")
                    psum_lg = state["lg2"][:, (tg % 2) * k:(tg % 2 + 1) * k]
                    lg_first = tg % 2 == 0
                else:
                    psum_lg = psL.tile([128, k], F32, name="lg")
                    lg_first = True
                last2 = state.pop(("last2", tg))
                ntr = (dc - 2) if last2 else (dc - HYB)
                if last2:
                    # chunks 6+7 as one DoubleRow pair from the HBM tensor
                    nc.tensor.matmul(
                        psum_lg,
                        ft7.rearrange("p (two m) -> p two m", two=2),
                        wt_sb[:, (dc - 2) * k:dc * k]
                        .rearrange("p (two n) -> p two n", two=2),
                        start=lg_first, stop=False,
                        perf_mode=mybir.MatmulPerfMode.DoubleRow,
                        skip_group_check=True,
                    )
                # HBM chunks first: they only wait on the row DMA, so the
                # copyback-dependent matmuls that follow fit the 4-slot PE
                # wait queue (no sequencer head-of-line block)
                for jj in range(0 if last2 else HYB // 2):  # DR pairs in ft7
                    nc.tensor.matmul(
                        psum_lg,
                        ft7[:, jj * 256:(jj + 1) * 256]
                        .rearrange("p (two m) -> p two m", two=2),
                        wt_sb[:, (ntr + 2 * jj) * k:(ntr + 2 * jj + 2) * k]
                        .rearrange("p (two n) -> p two n", two=2),
                        start=(lg_first and jj == 0), stop=False,
                        perf_mode=mybir.MatmulPerfMode.DoubleRow,
                        skip_group_check=True,
                    )
                if HYB % 2 and not last2:        # leftover single HBM chunk
                    nc.tensor.matmul(
                        psum_lg, ft7[:, (HYB - 1) * 128:HYB * 128],
                        wt_sb[:, (dc - 1) * k:dc * k],
                        start=(lg_first and HYB // 2 == 0), stop=False,
                        skip_group_check=True,
                    )
                for j2 in range(ntr // 2):       # DR pairs over PE chunks
                    nc.tensor.matmul(
                        psum_lg,
                        ft[:, j2 * 256:(j2 + 1) * 256]
                        .rearrange("p (two m) -> p two m", two=2),
                        wt_sb[:, j2 * 128:(j2 + 1) * 128]
                        .rearrange("p (two n) -> p two n", two=2),
                        start=(HYB == 0 and lg_first and j2 == 0),
                        stop=(ntr % 2 == 0 and j2 == ntr // 2 - 1),
                        perf_mode=mybir.MatmulPerfMode.DoubleRow,
                        skip_group_check=True,
                    )
                if ntr % 2:                      # leftover single PE chunk
                    nc.tensor.matmul(
                        psum_lg, ft[:, (ntr - 1) * 128:ntr * 128],
                        wt_sb[:, (ntr - 1) * k:ntr * k],
                        start=False, stop=True, skip_group_check=True,
                    )
                a_u = aupool.tile([128, k], BF16, name="au")
                nc.scalar.activation(
                    a_u, psum_lg, mybir.ActivationFunctionType.Exp,
                    bias=mask_sb[:, b * lt + t:b * lt + t + 1],
                )
                state[("au", tg)] = a_u

            def stage3(tg, chunks):
                """Pass B (U.T accumulation) for a subset of D chunks (+ s)."""
                b, t = divmod(tg, lt)
                if t == 0 and chunks[0] == 0:
                    rows[b] = (psU.tile([128, dc * k], F32, name="u"),
                               psS.tile([64, 1], F32, name="s"))
                psum_u, psum_s = rows[b]
                nat = state[("nat", tg)]
                a_u = state[("au", tg)]
                for di in chunks:
                    nc.tensor.matmul(
                        psum_u[:, di * k:(di + 1) * k],
                        nat[:, di * 128:(di + 1) * 128],
                        a_u,
                        start=(t == 0 and di == 0),
                        stop=(t == lt - 1 and di == dc - 1),
                        skip_group_check=True,
                    )
                if chunks[-1] == dc - 1:
                    nc.tensor.matmul(
                        psum_s, a_u, ones,
                        start=(t == 0), stop=(t == lt - 1),
                    )
                    state.pop(("nat", tg))
                    state.pop(("au", tg))
                    if t == lt - 1:
                        us_sb = outpool.tile([128, dc * k + 1], BF16, name="us")
                        hk = dc * k // 2
                        nc.scalar.copy(us_sb[:, 0:hk], psum_u[:, 0:hk])
                        nc.vector.tensor_copy(us_sb[:, hk:dc * k],
                                              psum_u[:, hk:dc * k])
                        nc.vector.memset(us_sb[64:128, dc * k:dc * k + 1], 0.0)
                        nc.vector.tensor_copy(us_sb[0:64, dc * k:dc * k + 1],
                                              psum_s)
                        nc.gpsimd.dma_start(out=out_us[b], in_=us_sb)

            S2, S3A, S3B = SKEWS
            lo = list(range(dc // 2))
            hi = list(range(dc // 2, dc))
            for tg in range(ntiles + S3B):
                if tg < ntiles:
                    stage1(tg)
                if tg == 0:
                    nc.scalar.dma_start(out=wt_sb, in_=wt_hbm[:])
                    nc.scalar.dma_start(out=mask_sb, in_=mask_hbm[:])
                if 0 <= tg - S2 < ntiles:
                    stage2(tg - S2)
                if 0 <= tg - S3A < ntiles:
                    stage3(tg - S3A, lo)
                if 0 <= tg - S3B < ntiles:
                    stage3(tg - S3B, hi)
    nc.compile()
    return nc


_NC_CACHE = {}


def _get_nc():
    key = (NSEG, FBUFS, PST_BUFS, PSU_BUFS, PSL_BUFS, LG_PAIR, SKEWS, HYB, ALTQ, ASPLIT, LASTROW2,
           os.environ.get("K_FTB", "8"), os.environ.get("K_AUB", "8"))
    if key not in _NC_CACHE:
        _NC_CACHE[key] = build_kernel()
    return _NC_CACHE[key]


def pack_host_inputs(feats, lens, assign_w, bpc=BPC, l=L, d=D, k=K, nseg=NSEG):
    """Host-side sharding + packing. Returns per-core input dicts."""
    np_bf16 = mybir.dt.np(BF16)
    np_fp8 = mybir.dt.np(FP8)
    lt = l // 128
    dc = d // 128
    seg_l = l // nseg
    spt = seg_l // 128

    wt_host = np.ascontiguousarray(assign_w.T).reshape(dc, 128, k)
    wt_p = np.ascontiguousarray(
        wt_host.transpose(1, 0, 2).reshape(128, dc * k)).astype(np_fp8)
    ident = np.eye(128, dtype=np_bf16)

    in_maps = []
    for i in range(NCORES):
        rows = feats[i * bpc:(i + 1) * bpc]                      # (bpc, L, D) f32
        b16 = rows.astype(np_bf16)
        # natural: [b, sg, p, jt*d + dd] = feats[b, sg*seg_l + jt*128 + p, dd]
        fnat = b16.reshape(bpc, nseg, spt, 128, d).transpose(0, 1, 3, 2, 4)
        fnat = np.ascontiguousarray(fnat).reshape(bpc, nseg, 128, spt * d)
        # transposed fp8 last HYB chunks, row-granular:
        # [b, p, t*HYB*128 + c*128 + ll] = feats[b, t*128+ll, (dc-HYB+c)*128+p]
        f7 = rows[:, :, (dc - HYB) * 128:].astype(np_fp8)        # (bpc, L, HYB*128)
        f7 = f7.reshape(bpc, lt, 128, HYB, 128).transpose(0, 4, 1, 3, 2)
        f7 = np.ascontiguousarray(f7).reshape(bpc, 128, lt * HYB * 128)

        lens_core = lens[i * bpc:(i + 1) * bpc]
        pos = (np.arange(lt)[None, :, None] * 128
               + np.arange(128)[None, None, :])                  # (1, lt, 128)
        m = np.where(pos < lens_core[:, None, None], 0.0, -1e30).astype(np.float32)
        mask_t = np.ascontiguousarray(m.transpose(2, 0, 1).reshape(128, bpc * lt))

        f67 = rows[bpc - 1, :, (dc - 2) * 128:].astype(np_fp8)   # (L, 256)
        f67 = f67.reshape(lt, 128, 2, 128).transpose(3, 0, 2, 1)
        f67 = np.ascontiguousarray(f67).reshape(128, lt * 256)
        im = {
            "fnat": fnat,
            "wt": wt_p,
            "mask_t": mask_t,
            "ident": ident,
        }
        if HYB:
            im["ft7"] = f7
        if LASTROW2:
            im["ft67"] = f67
        in_maps.append(im)
    return in_maps


def unpack_output(results, centroids, bpc=BPC, d=D, k=K):
    dc = d // 128
    out = np.empty((B, D), dtype=np.float32)
    for i in range(NCORES):
        us = results[i]["out_us"].astype(np.float32)   # (bpc, 128, dc*k+1)
        ut = us[:, :, 0:dc * k].reshape(bpc, 128, dc, k)
        u = np.ascontiguousarray(ut.transpose(0, 3, 2, 1)).reshape(bpc, k, d)
        s = us[:, 0:k, dc * k]               # (bpc, k)
        vlad = u / s[:, :, None] - centroids[None, :, :]
        o = vlad.min(axis=1)                 # (bpc, D)
        n = np.maximum(np.linalg.norm(o, axis=-1, keepdims=True), 1e-12)
        out[i * bpc:(i + 1) * bpc] = o / n
    return out


def kernel(feats, lens, assign_w, assign_b, centroids):
    feats = np.asarray(feats, dtype=np.float32)
    lens = np.asarray(lens, dtype=np.int32)
    assign_w = np.asarray(assign_w, dtype=np.float32)
    centroids = np.asarray(centroids, dtype=np.float32)

    nc = _get_nc()
    in_maps = pack_host_inputs(feats, lens, assign_w)
    # transient device errors (NRT_EXEC_UNIT_UNRECOVERABLE) recover on retry
    last_exc = None
    for _ in range(3):
        try:
            res = run_bass_kernel_spmd(nc, in_maps, core_ids=list(range(NCORES)))
            break
        except Exception as e:  # noqa: BLE001
            last_exc = e
    else:
        raise last_exc

    return unpack_output(res.results, centroids)

